# revision 5
# baseline (speedup 1.0000x reference)
"""Distributed Bass attention kernel for trn2 (8 NeuronCores), v2.

Problem: B=4,H=16,T=2048,D=128 attention w/ Q/K/V linear projections.
  qp = q@Wq.T+bq ; kp = k@Wk.T+bk ; vp = v@Wv.T+bv
  S = qp@kp.T/sqrt(128); S = where(mask==1, -1e-9, S); P=softmax(S); out = P@vp

Key identities (v2):
  - masked logit -1e-9 ~= 0  =>  masked P_unnorm = exp(0) = 1.
  - global shift C=ln(8): P' = exp(S*scale - C); masked entries = exactly
    0.125 (bf16-exact), cancels in softmax.
  - v2 ORDER CHANGE vs v1: exp runs FIRST, directly on the raw-logit PSUM
    (ScalarE, PSUM src = cheapest ACT form), then ONE DVE copy_predicated
    overwrites masked positions with the 0.125 constant (bf16 2x mode).
    This kills v1's 1x-rate fp32-PSUM mask-multiply (the DVE bottleneck)
    and all Scalar side-work.
  - proj biases bq/bk folded into PSUM via K=1 rank-1 matmuls
    (bias[1,128].T @ ones[1,512]); PSUM->SBUF casts on DVE (not ScalarE).
  - bv dropped on device: out = (P@vp)/l + bv applied on host (exact).
  - out row i ships raw O plus l (col 128 via vpx ones-column); host divides.

Sharding: 64 (b,h) slabs -> 8 per core (head/data parallel, no collectives).

Per-core dataflow (host pre-transposes+casts q/k/v to [d,t] bf16):
  - qpT[e,t] = wqt.T @ qT (+bq rank-1); kpT likewise (+bk)  [PE+DVE]
  - vp[t,e] via lhsT=vT tile, rhs=wvt (no bias); ones col appended
  - S TRANSPOSED in [128,1024] 2-bank PSUM: ST[j,i] = kpT_j.T @ qpT_i
  - Scalar: pt = exp(scale*ST - C) straight from PSUM -> bf16 SBUF
  - DVE: copy_predicated(pt_pair, maskT(bf16 0/1), 0.125-const) per jt-pair
  - AV: out[i,0:129] = sum_jt PT_tile.T @ vpx_tile (l in col 128)
  - epilogue: DVE copies AV PSUM -> ot8 SBUF, chunked DMA out
"""

import numpy as np
import ml_dtypes

import sys
sys.path.insert(0, "/opt/trn_rl_repo")

from concourse import bacc, bass, mybir
from concourse.tile import TileContext
from concourse.bass_utils import run_bass_kernel_spmd

B, H, T, D = 4, 16, 2048, 128
NCORES = 8
SPC = (B * H) // NCORES  # 8 slabs per core
NT = T // 128  # 16 j-tiles
IC = 1024  # i-chunk size
NCI = T // IC  # 2
SCALE = 1.0 / np.sqrt(D)
C_SHIFT = float(np.log(8.0))

F32 = mybir.dt.float32
BF16 = mybir.dt.bfloat16
U16 = mybir.dt.uint16
AF = mybir.ActivationFunctionType


def _build_nc():
    nc = bacc.Bacc(target_bir_lowering=False, trn_type="TRN2")

    qt_d = nc.declare_dram_parameter("qt", [SPC * 128, T], BF16, isOutput=False)
    kt_d = nc.declare_dram_parameter("kt", [SPC * 128, T], BF16, isOutput=False)
    vt_d = nc.declare_dram_parameter("vt", [SPC * 128, T], BF16, isOutput=False)
    mtb_d = nc.declare_dram_parameter("mtb", [T, T], U16, isOutput=False)
    wqt_d = nc.declare_dram_parameter("wqt", [D, D], BF16, isOutput=False)
    wkt_d = nc.declare_dram_parameter("wkt", [D, D], BF16, isOutput=False)
    wvt_d = nc.declare_dram_parameter("wvt", [D, D], BF16, isOutput=False)
    bq1_d = nc.declare_dram_parameter("bq1", [1, D], BF16, isOutput=False)
    bk1_d = nc.declare_dram_parameter("bk1", [1, D], BF16, isOutput=False)
    # out blocks: row = (s*NCI + ic)*128 + p, col = t*129 + e, e==128 is l
    out_d = nc.declare_dram_parameter(
        "out", [SPC * NCI * 128, IC // 128 * 129], F32, isOutput=True
    )

    with TileContext(nc) as tc:
        with (
            tc.tile_pool(name="const", bufs=1) as const_pool,
            tc.tile_pool(name="mmt", bufs=1) as mmt_pool,
            tc.tile_pool(name="qkvt", bufs=2) as qkvt_pool,
            tc.tile_pool(name="proj", bufs=2) as proj_pool,
            tc.tile_pool(name="vpx", bufs=2) as vpx_pool,
            tc.tile_pool(name="pt", bufs=2) as pt_pool,
            tc.tile_pool(name="fin", bufs=2) as fin_pool,
            tc.tile_pool(name="pj_ps", bufs=2, space="PSUM") as pjps_pool,
            tc.tile_pool(name="s_ps", bufs=2, space="PSUM") as sps_pool,
            tc.tile_pool(name="o_ps", bufs=2, space="PSUM") as ops_pool,
        ):
            # ---- constants; DMA order = first-use order (proj critical) ----
            wqt = const_pool.tile([128, 128], BF16, tag="wqt")
            nc.sync.dma_start(out=wqt[:, :], in_=wqt_d[:, :])
            wkt = const_pool.tile([128, 128], BF16, tag="wkt")
            nc.sync.dma_start(out=wkt[:, :], in_=wkt_d[:, :])

            # slab-0 q/k loads next so the proj->S pipeline starts asap
            qkv0 = [None, None, None]
            for idx, (name, srcd) in enumerate((("qT", qt_d), ("kT", kt_d))):
                t0 = qkvt_pool.tile([128, T], BF16, tag=name)
                nc.sync.dma_start(out=t0[:, :], in_=srcd[0:128, :])
                qkv0[idx] = t0

            bq1 = const_pool.tile([1, 128], BF16, tag="bq1")
            nc.sync.dma_start(out=bq1[:, :], in_=bq1_d[:, :])
            bk1 = const_pool.tile([1, 128], BF16, tag="bk1")
            nc.sync.dma_start(out=bk1[:, :], in_=bk1_d[:, :])

            # transposed mask (m==1 -> 1.0) bf16; i-chunk 0 of every j-tile
            # first (sme's ic=0 sweeps all 16 j-tiles), then i-chunk 1.
            mmtb = mmt_pool.tile([128, NT * T], U16, tag="mmtb")
            mmtb_v = mmtb[:, :].rearrange("p (j i) -> p j i", j=NT)
            mtb_v = mtb_d.rearrange("(j p) i -> p j i", p=128)
            nc.sync.dma_start(out=mmtb_v[:, :, 0:IC], in_=mtb_v[:, :, 0:IC])

            wvt = const_pool.tile([128, 128], BF16, tag="wvt")
            nc.sync.dma_start(out=wvt[:, :], in_=wvt_d[:, :])
            vT0 = qkvt_pool.tile([128, T], BF16, tag="vT")
            nc.sync.dma_start(out=vT0[:, :], in_=vt_d[0:128, :])
            qkv0[2] = vT0

            nc.sync.dma_start(out=mmtb_v[:, :, IC:T], in_=mtb_v[:, :, IC:T])

            ones512 = const_pool.tile([1, 512], BF16, tag="ones512")
            nc.vector.memset(ones512[:, :], 1.0)
            c0125 = const_pool.tile([128, 2 * IC], BF16, tag="c0125")
            nc.vector.memset(c0125[:, :], 0.125)
            negc = const_pool.tile([128, 1], F32, tag="negc")
            nc.vector.memset(negc[:, :], -C_SHIFT)

            # ---- software-pipelined slab phases ----
            def load(s):
                if s == 0:
                    return qkv0
                tiles = []
                for name, src in (("qT", qt_d), ("kT", kt_d), ("vT", vt_d)):
                    t = qkvt_pool.tile([128, T], BF16, tag=name)
                    nc.sync.dma_start(
                        out=t[:, :], in_=src[s * 128 : (s + 1) * 128, :]
                    )
                    tiles.append(t)
                return tiles

            def proj(qT, kT):
                qpT = proj_pool.tile([128, T], BF16, tag="qpT")
                kpT = proj_pool.tile([128, T], BF16, tag="kpT")
                for c in range(T // 512):
                    for srcT, w, b1, dst in (
                        (qT, wqt, bq1, qpT),
                        (kT, wkt, bk1, kpT),
                    ):
                        pps = pjps_pool.tile([128, 512], F32, tag="pj")
                        nc.tensor.matmul(
                            pps[:, :],
                            w[:, :],
                            srcT[:, c * 512 : (c + 1) * 512],
                            start=True,
                            stop=False,
                        )
                        # += bias[1,128].T @ ones[1,512]  (per-partition bias)
                        nc.tensor.matmul(
                            pps[:, :],
                            b1[:, :],
                            ones512[:, :],
                            start=False,
                            stop=True,
                        )
                        nc.vector.tensor_copy(
                            dst[:, c * 512 : (c + 1) * 512], pps[:, :]
                        )
                return qpT, kpT

            def vproj(vT):
                # vpx: 16 blocks [128(t), 129] bf16; col 128 = 1.0 (for l)
                vpx = vpx_pool.tile([128, NT * 130], BF16, tag="vpx")
                nc.gpsimd.memset(vpx[:, :], 1.0)
                vpxv = vpx[:, :].rearrange("p (j n) -> p j n", j=NT)  # n=130
                for b4 in range(NT // 4):
                    vps = pjps_pool.tile([128, 512], F32, tag="pj")
                    for t4 in range(4):
                        nc.tensor.matmul(
                            vps[:, t4 * 128 : (t4 + 1) * 128],
                            vT[:, (b4 * 4 + t4) * 128 : (b4 * 4 + t4 + 1) * 128],
                            wvt[:, :],
                            start=(t4 == 0),
                            stop=(t4 == 3),
                        )
                    nc.vector.tensor_copy(
                        vpxv[:, b4 * 4 : (b4 + 1) * 4, 0:128],
                        vps[:, :].rearrange("p (t n) -> p t n", t=4),
                    )
                return vpx, vpxv

            def sme(qpT, kpT, ic, pending_av=None):
                # S matmuls -> exp straight from PSUM -> masked-blend on DVE.
                # One AV group of the previous chunk is emitted between pairs
                # to keep PE/Scalar/DVE interleaved.
                i0 = ic * IC
                pt = pt_pool.tile([128, NT * IC], BF16, tag="pt")
                ptv3 = pt[:, :].rearrange("p (j i) -> p j i", j=NT)
                for tp in range(NT // 2):
                    for o in range(2):
                        jt = 2 * tp + o
                        st = sps_pool.tile([128, IC], F32, tag="s")
                        for h in range(IC // 512):
                            nc.tensor.matmul(
                                st[:, h * 512 : (h + 1) * 512],
                                kpT[:, jt * 128 : (jt + 1) * 128],
                                qpT[:, i0 + h * 512 : i0 + (h + 1) * 512],
                                start=True,
                                stop=True,
                            )
                        nc.scalar.activation(
                            pt[:, jt * IC : (jt + 1) * IC],
                            st[:, :],
                            AF.Exp,
                            bias=negc[:, :],
                            scale=float(SCALE),
                        )
                    # overwrite masked positions with exp(-C) = 0.125
                    nc.vector.copy_predicated(
                        ptv3[:, 2 * tp : 2 * tp + 2, :],
                        mmtb_v[:, 2 * tp : 2 * tp + 2, i0 : i0 + IC],
                        c0125[:, :].rearrange("p (j i) -> p j i", j=2),
                    )
                    if pending_av is not None:
                        pending_av(tp)
                return pt

            def make_av(s, ic, pt, vpxv):
                ptv = pt[:, :].rearrange("p (j i) -> p j i", j=NT)
                ot8 = fin_pool.tile([128, IC // 128 * 129], F32, tag="ot8")

                def emit(itl):
                    io = itl * 129
                    ops = ops_pool.tile([128, 129], F32, tag="o")
                    for jt in range(NT):
                        nc.tensor.matmul(
                            ops[:, :],
                            ptv[:, jt, itl * 128 : itl * 128 + 128],
                            vpxv[:, jt, 0:129],
                            start=(jt == 0),
                            stop=(jt == NT - 1),
                        )
                    # ship raw O plus l (col 128); normalize on host
                    nc.vector.tensor_copy(ot8[:, io : io + 129], ops[:, :])
                    if itl == IC // 128 - 1:
                        r0 = (s * NCI + ic) * 128
                        nc.sync.dma_start(
                            out=out_d[r0 : r0 + 128, :], in_=ot8[:, :]
                        )

                return emit

            pending = None
            for s in range(SPC):
                qT, kT, vT = load(s)
                qpT, kpT = proj(qT, kT)
                vpx, vpxv = vproj(vT)
                for ic in range(NCI):
                    pt = sme(qpT, kpT, ic, pending)
                    pending = make_av(s, ic, pt, vpxv)
            for tp in range(NT // 2):  # flush last chunk's AV groups
                pending(tp)
    if not nc.is_finalized():
        nc.finalize()
    return nc


_NC_CACHE = None


def kernel(q, k, v, mask, Wq, bq, Wk, bk, Wv, bv):
    global _NC_CACHE
    if _NC_CACHE is None:
        _NC_CACHE = _build_nc()
    nc = _NC_CACHE

    bf16 = ml_dtypes.bfloat16

    # host-side layout transforms (per-core slab-major, transposed, bf16)
    qf = np.asarray(q, np.float32).reshape(B * H, T, D)
    kf = np.asarray(k, np.float32).reshape(B * H, T, D)
    vf = np.asarray(v, np.float32).reshape(B * H, T, D)
    qt = np.ascontiguousarray(qf.transpose(0, 2, 1)).astype(bf16)  # [64,128,T]
    kt = np.ascontiguousarray(kf.transpose(0, 2, 1)).astype(bf16)
    vt = np.ascontiguousarray(vf.transpose(0, 2, 1)).astype(bf16)
    mtb = np.ascontiguousarray(
        np.asarray(mask, np.int32)[0, 0].T
    ).astype(np.uint16)  # 1 where masked
    wqt = np.ascontiguousarray(np.asarray(Wq, np.float32).T).astype(bf16)
    wkt = np.ascontiguousarray(np.asarray(Wk, np.float32).T).astype(bf16)
    wvt = np.ascontiguousarray(np.asarray(Wv, np.float32).T).astype(bf16)
    bq1 = np.asarray(bq, np.float32).reshape(1, D).astype(bf16)
    bk1 = np.asarray(bk, np.float32).reshape(1, D).astype(bf16)
    bvf = np.asarray(bv, np.float32).reshape(1, 1, 1, D)

    in_maps = []
    for c in range(NCORES):
        sl = slice(c * SPC, (c + 1) * SPC)
        in_maps.append(
            {
                "qt": np.ascontiguousarray(qt[sl].reshape(SPC * 128, T)),
                "kt": np.ascontiguousarray(kt[sl].reshape(SPC * 128, T)),
                "vt": np.ascontiguousarray(vt[sl].reshape(SPC * 128, T)),
                "mtb": mtb,
                "wqt": wqt,
                "wkt": wkt,
                "wvt": wvt,
                "bq1": bq1,
                "bk1": bk1,
            }
        )

    global _LAST_IN_MAPS
    _LAST_IN_MAPS = in_maps
    res = run_bass_kernel_spmd(nc, in_maps, core_ids=list(range(NCORES)))
    # out blocks: row=(s*NCI+ic)*128+p, col=t*129+e; col 128 of each block = l
    outs = [
        np.asarray(res.results[c]["out"]).reshape(SPC, NCI, 128, IC // 128, 129)
        for c in range(NCORES)
    ]
    raw = np.concatenate(outs, axis=0)  # [64, NCI, 128, 8, 129]
    full = raw[..., :D] / raw[..., D:]
    # i = ic*1024 + t*128 + p  ->  order (s, ic, t, p, e)
    full = full.transpose(0, 1, 3, 2, 4).reshape(B, H, T, D)
    return np.ascontiguousarray(full + bvf).astype(np.float32)


# revision 6
# speedup vs baseline: 1.2020x; 1.2020x over previous
"""Distributed Bass attention kernel for trn2 (8 NeuronCores), v3.

Problem: B=4,H=16,T=2048,D=128 attention w/ Q/K/V linear projections.
  qp = q@Wq.T+bq ; kp = k@Wk.T+bk ; vp = v@Wv.T+bv
  S = qp@kp.T/sqrt(128); S = where(mask==1, -1e-9, S); P=softmax(S); out = P@vp

Key identities (v3):
  - masked logit -1e-9 ~= 0  =>  masked P_unnorm = exp(0) = 1; with the
    global shift C=ln(8): P = exp(S*scale - C), masked P = 0.125 exactly.
  - THE BLEND TRICK: P_masked-blended = (U - 0.125)*w + 0.125 where
    U = exp(scale*S - C) (mask-oblivious) and w = 1-m. The affine part
    (U-0.125)*w is ONE fused DVE scalar_tensor_tensor op (4x-mode bf16);
    the +0.125 constant rides through the AV matmul as
    out += 0.125*colsum(vpx), where colsum(vpx) is computed EXACTLY on
    the host (tiny [129] vector per slab) and added during the PSUM
    drain (tensor_add instead of tensor_copy -- zero extra cost).
  - exp runs directly on the raw-logit PSUM (ScalarE, one [128,2048]
    ACT per jt-pair), so ScalarE does nothing but exp.
  - proj bias: DVE tensor_scalar(add bias[128,1]) fused into the
    PSUM->SBUF cast; no rank-1 matmuls, no ScalarE identities.
  - bv dropped on device: out = (P@vp)/l + bv applied on host (exact).
  - out row i ships raw O plus l (col 128 via vpx ones-column).

Sharding: 64 (b,h) slabs -> 8 per core (head/data parallel, no collectives).

Per-core engine budget (est): ScalarE 16x1850ns/slab = 237us; PE
S(110)+AV(121)+proj(14)+vproj(10) = 255us; DVE blend(76-144)+casts(64)+
drain(33) = 173-241us. Bottleneck PE/ScalarE ~255 -> target span ~270us.
"""

import numpy as np
import ml_dtypes

import sys
sys.path.insert(0, "/opt/trn_rl_repo")

from concourse import bacc, bass, mybir
from concourse.tile import TileContext
from concourse.bass_utils import run_bass_kernel_spmd

B, H, T, D = 4, 16, 2048, 128
NCORES = 8
SPC = (B * H) // NCORES  # 8 slabs per core
NT = T // 128  # 16 j-tiles
IC = 1024  # i-chunk size
NCI = T // IC  # 2
SCALE = 1.0 / np.sqrt(D)
C_SHIFT = float(np.log(8.0))

F32 = mybir.dt.float32
BF16 = mybir.dt.bfloat16
AF = mybir.ActivationFunctionType
ALU = mybir.AluOpType


def _build_nc():
    nc = bacc.Bacc(target_bir_lowering=False, trn_type="TRN2")

    qt_d = nc.declare_dram_parameter("qt", [SPC * 128, T], BF16, isOutput=False)
    kt_d = nc.declare_dram_parameter("kt", [SPC * 128, T], BF16, isOutput=False)
    vt_d = nc.declare_dram_parameter("vt", [SPC * 128, T], BF16, isOutput=False)
    # wtb = (1-mask).T : 1.0 at unmasked, 0.0 at masked
    wtb_d = nc.declare_dram_parameter("wtb", [T, T], BF16, isOutput=False)
    wqt_d = nc.declare_dram_parameter("wqt", [D, D], BF16, isOutput=False)
    wkt_d = nc.declare_dram_parameter("wkt", [D, D], BF16, isOutput=False)
    wvt_d = nc.declare_dram_parameter("wvt", [D, D], BF16, isOutput=False)
    bqc_d = nc.declare_dram_parameter("bqc", [D, 1], F32, isOutput=False)
    bkc_d = nc.declare_dram_parameter("bkc", [D, 1], F32, isOutput=False)
    # cs = 0.125*colsum(vpx) per slab, replicated on 128 partitions
    cs_d = nc.declare_dram_parameter("cs", [128, SPC * 130], F32, isOutput=False)
    # out blocks: row = (s*NCI + ic)*128 + p, col = t*129 + e, e==128 is l
    out_d = nc.declare_dram_parameter(
        "out", [SPC * NCI * 128, IC // 128 * 129], F32, isOutput=True
    )

    with TileContext(nc) as tc:
        with (
            tc.tile_pool(name="const", bufs=1) as const_pool,
            tc.tile_pool(name="mmt", bufs=1) as mmt_pool,
            tc.tile_pool(name="qkvt", bufs=2) as qkvt_pool,
            tc.tile_pool(name="proj", bufs=2) as proj_pool,
            tc.tile_pool(name="vpx", bufs=2) as vpx_pool,
            tc.tile_pool(name="scr", bufs=2) as scr_pool,
            tc.tile_pool(name="pt", bufs=2) as pt_pool,
            tc.tile_pool(name="fin", bufs=2) as fin_pool,
            tc.tile_pool(name="pj_ps", bufs=2, space="PSUM") as pjps_pool,
            tc.tile_pool(name="s_ps", bufs=1, space="PSUM") as sps_pool,
            tc.tile_pool(name="o_ps", bufs=2, space="PSUM") as ops_pool,
        ):
            # ---- constants; DMA order = first-use order (proj critical) ----
            wqt = const_pool.tile([128, 128], BF16, tag="wqt")
            nc.sync.dma_start(out=wqt[:, :], in_=wqt_d[:, :])
            wkt = const_pool.tile([128, 128], BF16, tag="wkt")
            nc.sync.dma_start(out=wkt[:, :], in_=wkt_d[:, :])

            # slab-0 q/k loads next so the proj->S pipeline starts asap
            qkv0 = [None, None, None]
            for idx, (name, srcd) in enumerate((("qT", qt_d), ("kT", kt_d))):
                t0 = qkvt_pool.tile([128, T], BF16, tag=name)
                nc.sync.dma_start(out=t0[:, :], in_=srcd[0:128, :])
                qkv0[idx] = t0

            bqc = const_pool.tile([128, 1], F32, tag="bqc")
            nc.sync.dma_start(out=bqc[:, :], in_=bqc_d[:, :])
            bkc = const_pool.tile([128, 1], F32, tag="bkc")
            nc.sync.dma_start(out=bkc[:, :], in_=bkc_d[:, :])

            # not-mask (1-m).T bf16; i-chunk 0 of every j-tile first
            # (sme's ic=0 sweeps all 16 j-tiles), then i-chunk 1.
            wtb = mmt_pool.tile([128, NT * T], BF16, tag="wtb")
            wtb_v = wtb[:, :].rearrange("p (j i) -> p j i", j=NT)
            wtb_sv = wtb_d.rearrange("(j p) i -> p j i", p=128)
            nc.sync.dma_start(out=wtb_v[:, :, 0:IC], in_=wtb_sv[:, :, 0:IC])

            wvt = const_pool.tile([128, 128], BF16, tag="wvt")
            nc.sync.dma_start(out=wvt[:, :], in_=wvt_d[:, :])
            vT0 = qkvt_pool.tile([128, T], BF16, tag="vT")
            nc.sync.dma_start(out=vT0[:, :], in_=vt_d[0:128, :])
            qkv0[2] = vT0
            cs = const_pool.tile([128, SPC * 130], F32, tag="cs")
            nc.sync.dma_start(out=cs[:, :], in_=cs_d[:, :])

            nc.sync.dma_start(out=wtb_v[:, :, IC:T], in_=wtb_sv[:, :, IC:T])

            negc = const_pool.tile([128, 1], F32, tag="negc")
            nc.vector.memset(negc[:, :], -C_SHIFT)

            # ---- software-pipelined slab phases ----
            def load(s):
                if s == 0:
                    return qkv0
                tiles = []
                for name, src in (("qT", qt_d), ("kT", kt_d), ("vT", vt_d)):
                    t = qkvt_pool.tile([128, T], BF16, tag=name)
                    nc.sync.dma_start(
                        out=t[:, :], in_=src[s * 128 : (s + 1) * 128, :]
                    )
                    tiles.append(t)
                return tiles

            def proj(qT, kT):
                qpT = proj_pool.tile([128, T], BF16, tag="qpT")
                kpT = proj_pool.tile([128, T], BF16, tag="kpT")
                for c in range(T // 512):
                    for srcT, w, bc, dst in (
                        (qT, wqt, bqc, qpT),
                        (kT, wkt, bkc, kpT),
                    ):
                        pps = pjps_pool.tile([128, 512], F32, tag="pj")
                        nc.tensor.matmul(
                            pps[:, :],
                            w[:, :],
                            srcT[:, c * 512 : (c + 1) * 512],
                            start=True,
                            stop=True,
                        )
                        # bias-add fused into the PSUM->SBUF bf16 cast
                        nc.vector.tensor_scalar(
                            dst[:, c * 512 : (c + 1) * 512],
                            pps[:, :],
                            bc[:, :],
                            None,
                            ALU.add,
                        )
                return qpT, kpT

            def vproj(vT):
                # vpx: 16 blocks [128(t), 129] bf16; col 128 = 1.0 (for l)
                vpx = vpx_pool.tile([128, NT * 130], BF16, tag="vpx")
                nc.gpsimd.memset(vpx[:, :], 1.0)
                vpxv = vpx[:, :].rearrange("p (j n) -> p j n", j=NT)  # n=130
                for b4 in range(NT // 4):
                    vps = pjps_pool.tile([128, 512], F32, tag="pj")
                    for t4 in range(4):
                        nc.tensor.matmul(
                            vps[:, t4 * 128 : (t4 + 1) * 128],
                            vT[:, (b4 * 4 + t4) * 128 : (b4 * 4 + t4 + 1) * 128],
                            wvt[:, :],
                            start=(t4 == 0),
                            stop=(t4 == 3),
                        )
                    nc.vector.tensor_copy(
                        vpxv[:, b4 * 4 : (b4 + 1) * 4, 0:128],
                        vps[:, :].rearrange("p (t n) -> p t n", t=4),
                    )
                return vpx, vpxv

            def sme(qpT, kpT, ic, pending_av=None):
                # S matmuls -> exp straight from PSUM -> fused masked-blend.
                # One AV group of the previous chunk is emitted between pairs
                # to keep PE/Scalar/DVE interleaved.
                i0 = ic * IC
                pt = pt_pool.tile([128, NT * IC], BF16, tag="pt")
                ptv3 = pt[:, :].rearrange("p (j i) -> p j i", j=NT)
                for tp in range(NT // 2):
                    st = sps_pool.tile([128, 2 * IC], F32, tag="s")
                    for o in range(2):
                        jt = 2 * tp + o
                        for h in range(IC // 512):
                            nc.tensor.matmul(
                                st[:, o * IC + h * 512 : o * IC + (h + 1) * 512],
                                kpT[:, jt * 128 : (jt + 1) * 128],
                                qpT[:, i0 + h * 512 : i0 + (h + 1) * 512],
                                start=True,
                                stop=True,
                            )
                    scr = scr_pool.tile([128, 2 * IC], BF16, tag="scr")
                    nc.scalar.activation(
                        scr[:, :],
                        st[:, :],
                        AF.Exp,
                        bias=negc[:, :],
                        scale=float(SCALE),
                    )
                    # pt = (U - 0.125) * w ; +0.125 rides via AV drain csum
                    nc.vector.scalar_tensor_tensor(
                        ptv3[:, 2 * tp : 2 * tp + 2, :],
                        scr[:, :].rearrange("p (j i) -> p j i", j=2),
                        0.125,
                        wtb_v[:, 2 * tp : 2 * tp + 2, i0 : i0 + IC],
                        ALU.subtract,
                        ALU.mult,
                    )
                    if pending_av is not None:
                        pending_av(tp)
                return pt

            def make_av(s, ic, pt, vpxv):
                ptv = pt[:, :].rearrange("p (j i) -> p j i", j=NT)
                ot8 = fin_pool.tile([128, IC // 128 * 129], F32, tag="ot8")

                def emit(itl):
                    io = itl * 129
                    ops = ops_pool.tile([128, 129], F32, tag="o")
                    for jt in range(NT):
                        nc.tensor.matmul(
                            ops[:, :],
                            ptv[:, jt, itl * 128 : itl * 128 + 128],
                            vpxv[:, jt, 0:129],
                            start=(jt == 0),
                            stop=(jt == NT - 1),
                        )
                    # drain + add the 0.125*colsum(vpx) constant (host-exact)
                    nc.vector.tensor_add(
                        ot8[:, io : io + 129],
                        ops[:, :],
                        cs[:, s * 130 : s * 130 + 129],
                    )
                    if itl == IC // 128 - 1:
                        r0 = (s * NCI + ic) * 128
                        nc.sync.dma_start(
                            out=out_d[r0 : r0 + 128, :], in_=ot8[:, :]
                        )

                return emit

            pending = None
            for s in range(SPC):
                qT, kT, vT = load(s)
                qpT, kpT = proj(qT, kT)
                vpx, vpxv = vproj(vT)
                for ic in range(NCI):
                    pt = sme(qpT, kpT, ic, pending)
                    pending = make_av(s, ic, pt, vpxv)
            for tp in range(NT // 2):  # flush last chunk's AV groups
                pending(tp)
    if not nc.is_finalized():
        nc.finalize()
    return nc


_NC_CACHE = None


def kernel(q, k, v, mask, Wq, bq, Wk, bk, Wv, bv):
    global _NC_CACHE
    if _NC_CACHE is None:
        _NC_CACHE = _build_nc()
    nc = _NC_CACHE

    bf16 = ml_dtypes.bfloat16

    # host-side layout transforms (per-core slab-major, transposed, bf16)
    qf = np.asarray(q, np.float32).reshape(B * H, T, D)
    kf = np.asarray(k, np.float32).reshape(B * H, T, D)
    vf = np.asarray(v, np.float32).reshape(B * H, T, D)
    qt = np.ascontiguousarray(qf.transpose(0, 2, 1)).astype(bf16)  # [64,128,T]
    kt = np.ascontiguousarray(kf.transpose(0, 2, 1)).astype(bf16)
    vt = np.ascontiguousarray(vf.transpose(0, 2, 1)).astype(bf16)
    wtb = np.ascontiguousarray(
        1.0 - np.asarray(mask, np.float32)[0, 0].T
    ).astype(bf16)  # 1.0 at unmasked
    wqt = np.ascontiguousarray(np.asarray(Wq, np.float32).T).astype(bf16)
    wkt = np.ascontiguousarray(np.asarray(Wk, np.float32).T).astype(bf16)
    wvt = np.ascontiguousarray(np.asarray(Wv, np.float32).T).astype(bf16)
    bqc = np.asarray(bq, np.float32).reshape(D, 1).copy()
    bkc = np.asarray(bk, np.float32).reshape(D, 1).copy()
    bvf = np.asarray(bv, np.float32).reshape(1, 1, 1, D)

    # 0.125 * colsum(vpx) per slab: cols 0..127 = 0.125*sum_t vp[t,:],
    # col 128 = 0.125*T (ones column), col 129 pad.
    Wvf = np.asarray(Wv, np.float32)
    vsum = vf.sum(axis=1)  # [64, 128]
    vpsum = vsum @ Wvf.T  # [64, 128]  (bv excluded; added on host at the end)

    in_maps = []
    for c in range(NCORES):
        sl = slice(c * SPC, (c + 1) * SPC)
        csl = np.zeros((SPC, 130), np.float32)
        csl[:, :D] = 0.125 * vpsum[sl]
        csl[:, D] = 0.125 * T
        cs_full = np.ascontiguousarray(
            np.broadcast_to(csl.reshape(1, SPC * 130), (128, SPC * 130))
        )
        in_maps.append(
            {
                "qt": np.ascontiguousarray(qt[sl].reshape(SPC * 128, T)),
                "kt": np.ascontiguousarray(kt[sl].reshape(SPC * 128, T)),
                "vt": np.ascontiguousarray(vt[sl].reshape(SPC * 128, T)),
                "wtb": wtb,
                "wqt": wqt,
                "wkt": wkt,
                "wvt": wvt,
                "bqc": bqc,
                "bkc": bkc,
                "cs": cs_full,
            }
        )

    global _LAST_IN_MAPS
    _LAST_IN_MAPS = in_maps
    res = run_bass_kernel_spmd(nc, in_maps, core_ids=list(range(NCORES)))
    # out blocks: row=(s*NCI+ic)*128+p, col=t*129+e; col 128 of each block = l
    outs = [
        np.asarray(res.results[c]["out"]).reshape(SPC, NCI, 128, IC // 128, 129)
        for c in range(NCORES)
    ]
    raw = np.concatenate(outs, axis=0)  # [64, NCI, 128, 8, 129]
    full = raw[..., :D] / raw[..., D:]
    # i = ic*1024 + t*128 + p  ->  order (s, ic, t, p, e)
    full = full.transpose(0, 1, 3, 2, 4).reshape(B, H, T, D)
    return np.ascontiguousarray(full + bvf).astype(np.float32)
